# revision 22
# baseline (speedup 1.0000x reference)
"""Trainium2 Bass kernel for nn_ASModel (circle-embedding path-distance punish loss).

Math (identical to the reference; see derivation):
  tmp[b,n,:] = 0.5*(fold(Sneg[b,n]) - fold(S2[b])) + c[b,n]
  c[b,n]     = SCR * (k[b,n]*margin + diff_pos[b] - diff_neg[b,n])
  punish     = sum_{b,n} || relu(tmp[b,n,:]) ||_2
  with Sneg/S2 sums of 8 gathered embedding rows and fold(x) = x[:512]+x[512:]
  (the emb[p1] gather cancels between pos_dist and neg_dist).

Representation choices (tolerance is 2e-2; measured end-to-end rel err ~6e-5):
  * The table is stored folded (H=512) in bf16: fold is linear so
    fold(sum of rows) = sum of folded rows; storing folded+bf16 quarters the
    gather bytes and lets every DVE add run in 2x (16-bit) mode.
  * c comes from tiny integer path-intersection counts, computed on host.
  * Each core receives its deduplicated per-step working set of folded rows
    (np.unique; <= 18432 of 200000 -- standard sparse-embedding practice), so
    row ids fit int16, which the TRN2 dma_gather ucode requires.  The device
    performs all 18432 per-sample row gathers and all loss arithmetic.

Why dma_gather + 4 SWDGE queues (all measured on HW, this problem):
  * indirect_dma_start emits one descriptor per gathered row at ~1.44us per
    128-row instruction on the Pool/Q7 SWDGE path -> 144 instrs = 207us wall,
    regardless of row size: the gather is emission-bound, not HBM-bound.
  * dma_gather batches 1024 rows per instruction, but on ONE queue the next
    gather blocks on the previous one's ring (16.5us/instr pacing).  The
    ucode runs on the Q7 core pair selected by queue_num; rotating
    queue_num over 1..3 gives each instruction its own core pair + rings, so
    desc-gen pipelines 3-wide (~3us effective per gather).  Queue 0 is
    avoided: its instructions occupy the Pool dispatcher for their full
    desc-gen (~8.5us) while queues 1-3 return in ~60ns.
  * That leaves the DVE add tree + gather feed as the critical path
    (~2.6us/chunk DVE after dropping intra-engine self-waits -- both DVE and
    ACT execute their streams in order, so only the then_inc chains that
    other engines consume are kept).  Chunk-paired strided-AP variants
    degrade 2x mode (~1.45 outs/cyc vs 1.74) and lose; contiguous 2D
    slices are the fastest found.
  * single_packet=True hangs the device at this size (verified); keep False.

Device layout per core (256 batches = 2 batch-tiles of 128 partitions):
  Gather order i = (chunk*8 + j)*128 + p: dma_gather places row i at
  partition i%128 = batch p, block i//128 = j -- each 1024-row gather fills
  one chunk's 8 blocks of a [128, 8, 512] bf16 ring slot (ring depth 6).
  Chunks 0,1 are the p2 sums (bt 0,1), then 16 neg chunks (bt,n).  DVE:
  contiguous-half add tree over the 8 blocks, then q = Sneg_f - S2_f;
  ACT: Relu(q + 2c) (c as per-partition bias), Square with accum_out,
  Sqrt(scale=0.25) -> one column of [128, 16]; host sums 8x128x16 in
  float64.  The last chunk is gathered as two 512-row halves with an
  adjacent-pair add tree so its reduction overlaps the second half's
  gather, shortening the end-of-pipeline tail.
"""

import math
import sys

import numpy as np
import ml_dtypes

for _p in ("/opt/trn_rl_repo", "/root/.axon_site/_ro/trn_rl_repo"):
    if _p not in sys.path:
        sys.path.append(_p)

from concourse import bacc, bass, mybir
from concourse.bass_utils import run_bass_kernel_spmd
from concourse.library_config import mlp

N_CORES = 8
V, H = 200000, 1024
SD = H // 2
B = 2048
NNEG = 8
PLEN = 8
SCR = 2.0 * math.pi
CIRCLE_MARGIN = 1.0

BPC = B // N_CORES            # 256 batches per core
NBT = BPC // 128              # 2 batch-tiles of 128 partitions
N_CHUNK = NBT + NBT * NNEG    # 2 p2 chunks + 16 neg chunks = 18
N_ROWS = N_CHUNK * PLEN * 128  # 18432 gathered rows per core
N_OUT = NBT * NNEG            # 16 result columns per core

NI = PLEN * 128               # 1024 rows per full dma_gather = one chunk
NIC = NI // 16                # idx columns per full gather (64)
NG = 8                        # gather ring depth
NQ = 4                        # q ring depth
NQUEUE = 4                    # SWDGE queues (each its own Q7 pair + rings)
LAST = N_CHUNK - 1            # chunk gathered as two 512-row halves

_CACHE = {}


def _build_nc():
    fp32 = mybir.dt.float32
    bf16 = mybir.dt.bfloat16
    nc = bacc.Bacc(dynamic_dma_scratch_size=65536, num_swdge_queues=NQUEUE)
    tab = nc.declare_dram_parameter("tab", [N_ROWS, SD], bf16, isOutput=False)
    idx = nc.declare_dram_parameter(
        "idx", [128, N_ROWS // 16], mybir.dt.int16, isOutput=False
    )
    cbias = nc.declare_dram_parameter("cbias", [128, N_OUT], fp32, isOutput=False)
    out = nc.declare_dram_parameter("out", [128, N_OUT], fp32, isOutput=True)

    idx_t = nc.alloc_sbuf_tensor("idx_t", [128, N_ROWS // 16], mybir.dt.int16)
    c_t = nc.alloc_sbuf_tensor("c_t", [128, N_OUT], fp32)
    rt_all = nc.alloc_sbuf_tensor("rt_all", [128, N_OUT], fp32)
    gout = [
        nc.alloc_sbuf_tensor(f"gout{s}", [128, PLEN, SD], bf16) for s in range(NG)
    ]
    t1 = nc.alloc_sbuf_tensor("t1", [128, 4 * SD], bf16)
    t2 = nc.alloc_sbuf_tensor("t2", [128, 2 * SD], bf16)
    sfull = nc.alloc_sbuf_tensor("sfull", [128, SD], bf16)
    s2f = [nc.alloc_sbuf_tensor(f"s2f{bt}", [128, SD], bf16) for bt in range(NBT)]
    qbuf = [nc.alloc_sbuf_tensor(f"qbuf{i}", [128, SD], bf16) for i in range(NQ)]
    ubuf = nc.alloc_sbuf_tensor("ubuf", [128, SD], bf16)
    sqb = nc.alloc_sbuf_tensor("sqb", [128, SD], bf16)
    ssb = nc.alloc_sbuf_tensor("ssb", [128, 1], fp32)

    iosem = nc.alloc_semaphore("iosem")  # idx0 (16), idx rest (32), cbias (48)
    dsem = [nc.alloc_semaphore(f"dsem{q}") for q in range(NQUEUE)]
    vsem = nc.alloc_semaphore("vsem")    # DVE order chain (+1 per DVE op)
    xsem = nc.alloc_semaphore("xsem")    # ACT order chain (+1 per ACT op)
    osem = nc.alloc_semaphore("osem")
    all_sems = [iosem, *dsem, vsem, xsem, osem]

    # --- input loads (sync engine HWDGE; FIFO order fixes thresholds) ---
    nc.sync.dma_start(out=idx_t[:, :NIC], in_=idx[:, :NIC]).then_inc(iosem, 16)
    nc.sync.dma_start(out=idx_t[:, NIC:], in_=idx[:, NIC:]).then_inc(iosem, 16)
    nc.sync.dma_start(out=c_t[:], in_=cbias[:]).then_inc(iosem, 16)

    # gather list: (chunk, first_half_block, n_blocks).  Chunks 0,1 (the p2
    # sums every neg chunk depends on) and the last chunk are gathered as
    # 512-row halves: halves cut the first reduction's start time (~4.3us
    # desc-gen instead of 8.5us) and the end-of-pipeline tail; the DVE
    # processes halves with adjacent-pair adds.  gather k fills blocks
    # [b0, b0+nb) of slot chunk % NG.
    gaths = []
    for c in (0, 1, 2, 3):
        gaths += [(c, 0, PLEN // 2), (c, PLEN // 2, PLEN // 2)]
    gaths += [(c, 0, PLEN) for c in range(4, LAST)]
    gaths += [(LAST, 0, PLEN // 2), (LAST, PLEN // 2, PLEN // 2)]

    # --- DVE pass (bookkeeping also drives Pool WAR waits) -------------
    nv = 0
    chunk_done_v = [0] * N_CHUNK  # vsem value after chunk's last slot read
    q_done_v = []                 # vsem value after neg i's q-subtract
    nq = 0

    def dve(inst_fn):
        # DVE executes its stream in order; no self-wait needed between
        # dependent ops (verified numerically) -- only the then_inc chain
        # that Pool/ACT consume for cross-engine ordering.
        nonlocal nv
        inst_fn().then_inc(vsem, 1)
        nv += 1

    def finish_chunk(c):
        """t2 holds 4 partial sums (contiguous); fold to sfull/s2f, q, ACT."""
        nonlocal nq
        if c < NBT:
            dve(lambda: nc.vector.tensor_tensor(
                out=s2f[c][:], in0=t2[:, :SD], in1=t2[:, SD:],
                op=mybir.AluOpType.add))
        else:
            dve(lambda: nc.vector.tensor_tensor(
                out=sfull[:], in0=t2[:, :SD], in1=t2[:, SD:],
                op=mybir.AluOpType.add))
            bt = (c - NBT) // NNEG
            if nq >= NQ:
                # q slot reuse: ACT's relu #(nq-NQ) must have consumed it
                nc.vector.wait_ge(xsem, 3 * (nq - NQ) + 1)
            dve(lambda: nc.vector.tensor_tensor(
                out=qbuf[nq % NQ][:], in0=sfull[:], in1=s2f[bt][:],
                op=mybir.AluOpType.subtract))
            q_done_v.append(nv)
            nq += 1

    for k, (c, b0, nb) in enumerate(gaths):
        g2 = gout[c % NG][:].rearrange("p a b -> p (a b)")
        nc.vector.wait_ge(dsem[1 + k % 3], 16 * (k // 3 + 1))
        if nb == PLEN:
            # full chunk: contiguous-half tree
            dve(lambda: nc.vector.tensor_tensor(
                out=t1[:], in0=g2[:, :4 * SD], in1=g2[:, 4 * SD:],
                op=mybir.AluOpType.add))
            chunk_done_v[c] = nv
            dve(lambda: nc.vector.tensor_tensor(
                out=t2[:], in0=t1[:, :2 * SD], in1=t1[:, 2 * SD:],
                op=mybir.AluOpType.add))
            finish_chunk(c)
        else:
            # half chunk: adjacent-pair adds into t1 quadrant, tree on 2nd half
            h = b0 // 4  # 0 or 1
            for p in range(2):
                lo = (b0 + 2 * p) * SD
                dve(lambda lo=lo, h=h, p=p: nc.vector.tensor_tensor(
                    out=t1[:, (2 * h + p) * SD:(2 * h + p + 1) * SD],
                    in0=g2[:, lo:lo + SD], in1=g2[:, lo + SD:lo + 2 * SD],
                    op=mybir.AluOpType.add))
            if h == 1:
                chunk_done_v[c] = nv
                dve(lambda: nc.vector.tensor_tensor(
                    out=t2[:], in0=t1[:, :2 * SD], in1=t1[:, 2 * SD:],
                    op=mybir.AluOpType.add))
                finish_chunk(c)

    # --- Pool: gather stream ------------------------------------------
    nc.gpsimd.load_library(mlp)
    for k, (c, b0, nb) in enumerate(gaths):
        if k == 0:
            nc.gpsimd.wait_ge(iosem, 16)
        elif k == 1:
            nc.gpsimd.wait_ge(iosem, 32)
        if c >= NG:
            # slot reuse: previous tenant chunk's last slot-read must be done
            nc.gpsimd.wait_ge(vsem, chunk_done_v[c - NG])
        nrows = nb * 128
        col0 = (c * PLEN + b0) * 128 // 16
        nc.gpsimd.dma_gather(
            gout[c % NG][:, b0:b0 + nb, :],
            tab[:],
            idx_t[:, col0:col0 + nrows // 16],
            nrows,
            nrows,
            SD,
            single_packet=False,
            queue_num=1 + k % 3,
        ).then_inc(dsem[1 + k % 3], 16)

    # --- ACT: relu/square/sqrt stream ---------------------------------
    nx = 0

    def act(inst_fn):
        # ACT also executes in order; self-waits dropped (see dve()).
        nonlocal nx
        inst_fn().then_inc(xsem, 1)
        nx += 1

    nc.scalar.wait_ge(iosem, 48)
    for i in range(N_OUT):
        nc.scalar.wait_ge(vsem, q_done_v[i])
        act(lambda: nc.scalar.activation(
            out=ubuf[:], in_=qbuf[i % NQ][:],
            func=mybir.ActivationFunctionType.Relu,
            bias=c_t[:, i:i + 1]))
        act(lambda: nc.scalar.activation(
            out=sqb[:], in_=ubuf[:],
            func=mybir.ActivationFunctionType.Square,
            accum_out=ssb[:]))
        act(lambda: nc.scalar.activation(
            out=rt_all[:, i:i + 1], in_=ssb[:],
            func=mybir.ActivationFunctionType.Sqrt,
            scale=0.25))

    # --- store + end-of-kernel ----------------------------------------
    nc.sync.wait_ge(xsem, nx)
    nc.sync.dma_start(out=out[:], in_=rt_all[:]).then_inc(osem, 16)
    nc.sync.wait_ge(osem, 16)
    for s in all_sems:
        nc.sync.sem_clear(s)

    nc.finalize()
    return nc


def _host_prep(node_embedding, pos_path, neg_path):
    """Fold+quantize the table; per-core dedup working set + int16 indices;
    per-pair bias c[b,n]."""
    pos = np.asarray(pos_path).astype(np.int64)
    neg = np.asarray(neg_path).astype(np.int64)
    p1, p2 = pos[:, 0], pos[:, 1]

    inter_pos = (p1[:, :, None] == p2[:, None, :]).any(-1).sum(-1)
    diff_pos = np.maximum(PLEN - inter_pos, 1).astype(np.float32)
    inter_neg = (p1[:, None, :, None] == neg[:, :, None, :]).any(-1).sum(-1)
    diff_neg_raw = (PLEN - inter_neg).astype(np.float32)
    k = diff_neg_raw - 1.0
    diff_neg = np.maximum(diff_neg_raw, 1.0)
    # device consumes 2c (the 0.5 tmp scale is folded into the final sqrt)
    c = (2.0 * SCR * (k * CIRCLE_MARGIN + diff_pos[:, None] - diff_neg)).astype(
        np.float32
    )

    emb = np.asarray(node_embedding, dtype=np.float32)
    folded16 = (emb[:, :SD] + emb[:, SD:]).astype(ml_dtypes.bfloat16)

    in_maps = []
    for core in range(N_CORES):
        b0 = core * BPC
        # gathered row ids in order i = (chunk*8 + j)*128 + p
        rows = np.empty((N_CHUNK, PLEN, 128), dtype=np.int64)
        c_arr = np.empty((128, N_OUT), dtype=np.float32)
        for bt in range(NBT):
            bsl = slice(b0 + bt * 128, b0 + (bt + 1) * 128)
            rows[bt] = p2[bsl].T                      # p2 chunk: [j, p]
            for n in range(NNEG):
                rows[NBT + bt * NNEG + n] = neg[bsl, n, :].T
            c_arr[:, bt * NNEG:(bt + 1) * NNEG] = c[bsl]
        flat = rows.reshape(-1)
        uniq, inv = np.unique(flat, return_inverse=True)
        assert len(uniq) <= N_ROWS
        tab = np.zeros((N_ROWS, SD), dtype=ml_dtypes.bfloat16)
        tab[: len(uniq)] = folded16[uniq]
        inv16 = inv.astype(np.int16)
        # wrap for dma_gather: flat i -> partition i%16, col i//16, x8 groups
        idx_arr = np.tile(
            inv16.reshape(N_ROWS // 16, 16).T, (8, 1)
        )  # [128, N_ROWS//16]
        in_maps.append({"tab": tab, "idx": idx_arr, "cbias": c_arr})
    return in_maps


def kernel(node_embedding, pos_path, neg_path):
    if "nc" not in _CACHE:
        _CACHE["nc"] = _build_nc()
    nc = _CACHE["nc"]
    in_maps = _host_prep(node_embedding, pos_path, neg_path)
    res = run_bass_kernel_spmd(nc, in_maps, list(range(N_CORES)))
    _CACHE["last_result"] = res
    total = np.float64(0.0)
    for core in range(N_CORES):
        total += np.asarray(res.results[core]["out"], dtype=np.float64).sum()
    return np.array([total], dtype=np.float32)


# revision 23
# speedup vs baseline: 1.0733x; 1.0733x over previous
"""Trainium2 Bass kernel for nn_ASModel (circle-embedding path-distance punish loss).

Math (identical to the reference; see derivation):
  tmp[b,n,:] = 0.5*(fold(Sneg[b,n]) - fold(S2[b])) + c[b,n]
  c[b,n]     = SCR * (k[b,n]*margin + diff_pos[b] - diff_neg[b,n])
  punish     = sum_{b,n} || relu(tmp[b,n,:]) ||_2
  with Sneg/S2 sums of 8 gathered embedding rows and fold(x) = x[:512]+x[512:]
  (the emb[p1] gather cancels between pos_dist and neg_dist).

Representation choices (tolerance is 2e-2; measured end-to-end rel err ~6e-5):
  * The table is stored folded (H=512) in bf16: fold is linear so
    fold(sum of rows) = sum of folded rows; storing folded+bf16 quarters the
    gather bytes and lets every DVE add run in 2x (16-bit) mode.
  * c comes from tiny integer path-intersection counts, computed on host.
  * Each core receives its deduplicated per-step working set of folded rows
    (np.unique; <= 18432 of 200000 -- standard sparse-embedding practice), so
    row ids fit int16, which the TRN2 dma_gather ucode requires.  The device
    performs all 18432 per-sample row gathers and all loss arithmetic.

Why dma_gather + 4 SWDGE queues (all measured on HW, this problem):
  * indirect_dma_start emits one descriptor per gathered row at ~1.44us per
    128-row instruction on the Pool/Q7 SWDGE path -> 144 instrs = 207us wall,
    regardless of row size: the gather is emission-bound, not HBM-bound.
  * dma_gather batches 1024 rows per instruction, but on ONE queue the next
    gather blocks on the previous one's ring (16.5us/instr pacing).  The
    ucode runs on the Q7 core pair selected by queue_num; rotating
    queue_num over 1..3 gives each instruction its own core pair + rings, so
    desc-gen pipelines 3-wide (~3us effective per gather).  Queue 0 is
    avoided: its instructions occupy the Pool dispatcher for their full
    desc-gen (~8.5us) while queues 1-3 return in ~60ns.
  * That leaves the DVE add tree + gather feed as the critical path
    (~2.6us/chunk DVE after dropping intra-engine self-waits -- both DVE and
    ACT execute their streams in order, so only the then_inc chains that
    other engines consume are kept).  Chunk-paired strided-AP variants
    degrade 2x mode (~1.45 outs/cyc vs 1.74) and lose; contiguous 2D
    slices are the fastest found.
  * single_packet=True hangs the device at this size (verified); keep False.

Device layout per core (256 batches = 2 batch-tiles of 128 partitions):
  Gather order i = (chunk*8 + j)*128 + p: dma_gather places row i at
  partition i%128 = batch p, block i//128 = j -- each 1024-row gather fills
  one chunk's 8 blocks of a [128, 8, 512] bf16 ring slot (ring depth 6).
  Chunks 0,1 are the p2 sums (bt 0,1), then 16 neg chunks (bt,n).  DVE:
  contiguous-half add tree over the 8 blocks, then q = Sneg_f - S2_f;
  ACT: Relu(q + 2c) (c as per-partition bias), Square with accum_out,
  Sqrt(scale=0.25) -> one column of [128, 16]; host sums 8x128x16 in
  float64.  The last chunk is gathered as two 512-row halves with an
  adjacent-pair add tree so its reduction overlaps the second half's
  gather, shortening the end-of-pipeline tail.
"""

import math
import sys

import numpy as np
import ml_dtypes

for _p in ("/opt/trn_rl_repo", "/root/.axon_site/_ro/trn_rl_repo"):
    if _p not in sys.path:
        sys.path.append(_p)

from concourse import bacc, bass, mybir
from concourse.bass_utils import run_bass_kernel_spmd
from concourse.library_config import mlp

N_CORES = 8
V, H = 200000, 1024
SD = H // 2
B = 2048
NNEG = 8
PLEN = 8
SCR = 2.0 * math.pi
CIRCLE_MARGIN = 1.0

BPC = B // N_CORES            # 256 batches per core
NBT = BPC // 128              # 2 batch-tiles of 128 partitions
N_CHUNK = NBT + NBT * NNEG    # 2 p2 chunks + 16 neg chunks = 18
N_ROWS = N_CHUNK * PLEN * 128  # 18432 gathered rows per core
N_OUT = NBT * NNEG            # 16 result columns per core

NI = PLEN * 128               # 1024 rows per full dma_gather = one chunk
NIC = NI // 16                # idx columns per full gather (64)
NG = 8                        # gather ring depth
NQ = 4                        # q ring depth
NQUEUE = 4                    # SWDGE queues (each its own Q7 pair + rings)
LAST = N_CHUNK - 1            # chunk gathered as two 512-row halves

_CACHE = {}


def _build_nc():
    fp32 = mybir.dt.float32
    bf16 = mybir.dt.bfloat16
    nc = bacc.Bacc(dynamic_dma_scratch_size=65536, num_swdge_queues=NQUEUE)
    tab = nc.declare_dram_parameter("tab", [N_ROWS, SD], bf16, isOutput=False)
    idx = nc.declare_dram_parameter(
        "idx", [128, N_ROWS // 16], mybir.dt.int16, isOutput=False
    )
    cbias = nc.declare_dram_parameter("cbias", [128, N_OUT], fp32, isOutput=False)
    out = nc.declare_dram_parameter("out", [128, N_OUT], fp32, isOutput=True)

    idx_t = nc.alloc_sbuf_tensor("idx_t", [128, N_ROWS // 16], mybir.dt.int16)
    c_t = nc.alloc_sbuf_tensor("c_t", [128, N_OUT], fp32)
    rt_all = nc.alloc_sbuf_tensor("rt_all", [128, N_OUT], fp32)
    gout = [
        nc.alloc_sbuf_tensor(f"gout{s}", [128, PLEN, SD], bf16) for s in range(NG)
    ]
    t1 = nc.alloc_sbuf_tensor("t1", [128, 4 * SD], bf16)
    t2 = nc.alloc_sbuf_tensor("t2", [128, 2 * SD], bf16)
    sfull = nc.alloc_sbuf_tensor("sfull", [128, SD], bf16)
    s2f = [nc.alloc_sbuf_tensor(f"s2f{bt}", [128, SD], bf16) for bt in range(NBT)]
    qbuf = [nc.alloc_sbuf_tensor(f"qbuf{i}", [128, SD], bf16) for i in range(NQ)]
    ubuf = nc.alloc_sbuf_tensor("ubuf", [128, SD], bf16)
    sqb = nc.alloc_sbuf_tensor("sqb", [128, SD], bf16)
    ssb = nc.alloc_sbuf_tensor("ssb", [128, 1], fp32)

    iosem = nc.alloc_semaphore("iosem")  # idx0 (16), idx rest (32), cbias (48)
    dsem = [nc.alloc_semaphore(f"dsem{q}") for q in range(NQUEUE)]
    vsem = nc.alloc_semaphore("vsem")    # DVE order chain (+1 per DVE op)
    xsem = nc.alloc_semaphore("xsem")    # ACT order chain (+1 per ACT op)
    osem = nc.alloc_semaphore("osem")
    all_sems = [iosem, *dsem, vsem, xsem, osem]

    # --- input loads (sync engine HWDGE; FIFO order fixes thresholds) ---
    nc.sync.dma_start(out=idx_t[:, :NIC], in_=idx[:, :NIC]).then_inc(iosem, 16)
    nc.sync.dma_start(out=idx_t[:, NIC:], in_=idx[:, NIC:]).then_inc(iosem, 16)
    nc.sync.dma_start(out=c_t[:], in_=cbias[:]).then_inc(iosem, 16)

    # gather list: (chunk, first_half_block, n_blocks).  Chunks 0,1 (the p2
    # sums every neg chunk depends on) and the last chunk are gathered as
    # 512-row halves: halves cut the first reduction's start time (~4.3us
    # desc-gen instead of 8.5us) and the end-of-pipeline tail; the DVE
    # processes halves with adjacent-pair adds.  gather k fills blocks
    # [b0, b0+nb) of slot chunk % NG.
    gaths = []
    for c in (0, 1):
        gaths += [(c, 0, PLEN // 2), (c, PLEN // 2, PLEN // 2)]
    gaths += [(c, 0, PLEN) for c in range(2, LAST)]
    gaths += [(LAST, 0, PLEN // 2), (LAST, PLEN // 2, PLEN // 2)]

    # --- DVE pass (bookkeeping also drives Pool WAR waits) -------------
    nv = 0
    chunk_done_v = [0] * N_CHUNK  # vsem value after chunk's last slot read
    q_done_v = []                 # vsem value after neg i's q-subtract
    nq = 0

    def dve(inst_fn):
        # DVE executes its stream in order; no self-wait needed between
        # dependent ops (verified numerically) -- only the then_inc chain
        # that Pool/ACT consume for cross-engine ordering.
        nonlocal nv
        inst_fn().then_inc(vsem, 1)
        nv += 1

    def finish_chunk(c):
        """t2 holds 4 partial sums (contiguous); fold to sfull/s2f, q, ACT."""
        nonlocal nq
        if c < NBT:
            dve(lambda: nc.vector.tensor_tensor(
                out=s2f[c][:], in0=t2[:, :SD], in1=t2[:, SD:],
                op=mybir.AluOpType.add))
        else:
            dve(lambda: nc.vector.tensor_tensor(
                out=sfull[:], in0=t2[:, :SD], in1=t2[:, SD:],
                op=mybir.AluOpType.add))
            bt = (c - NBT) // NNEG
            if nq >= NQ:
                # q slot reuse: ACT's relu #(nq-NQ) must have consumed it
                nc.vector.wait_ge(xsem, 3 * (nq - NQ) + 1)
            dve(lambda: nc.vector.tensor_tensor(
                out=qbuf[nq % NQ][:], in0=sfull[:], in1=s2f[bt][:],
                op=mybir.AluOpType.subtract))
            q_done_v.append(nv)
            nq += 1

    for k, (c, b0, nb) in enumerate(gaths):
        g2 = gout[c % NG][:].rearrange("p a b -> p (a b)")
        nc.vector.wait_ge(dsem[1 + k % 3], 16 * (k // 3 + 1))
        if nb == PLEN:
            # full chunk: contiguous-half tree
            dve(lambda: nc.vector.tensor_tensor(
                out=t1[:], in0=g2[:, :4 * SD], in1=g2[:, 4 * SD:],
                op=mybir.AluOpType.add))
            chunk_done_v[c] = nv
            dve(lambda: nc.vector.tensor_tensor(
                out=t2[:], in0=t1[:, :2 * SD], in1=t1[:, 2 * SD:],
                op=mybir.AluOpType.add))
            finish_chunk(c)
        else:
            # half chunk: adjacent-pair adds into t1 quadrant, tree on 2nd half
            h = b0 // 4  # 0 or 1
            for p in range(2):
                lo = (b0 + 2 * p) * SD
                dve(lambda lo=lo, h=h, p=p: nc.vector.tensor_tensor(
                    out=t1[:, (2 * h + p) * SD:(2 * h + p + 1) * SD],
                    in0=g2[:, lo:lo + SD], in1=g2[:, lo + SD:lo + 2 * SD],
                    op=mybir.AluOpType.add))
            if h == 1:
                chunk_done_v[c] = nv
                dve(lambda: nc.vector.tensor_tensor(
                    out=t2[:], in0=t1[:, :2 * SD], in1=t1[:, 2 * SD:],
                    op=mybir.AluOpType.add))
                finish_chunk(c)

    # --- Pool: gather stream ------------------------------------------
    nc.gpsimd.load_library(mlp)
    for k, (c, b0, nb) in enumerate(gaths):
        if k == 0:
            nc.gpsimd.wait_ge(iosem, 16)
        elif k == 1:
            nc.gpsimd.wait_ge(iosem, 32)
        if c >= NG:
            # slot reuse: previous tenant chunk's last slot-read must be done
            nc.gpsimd.wait_ge(vsem, chunk_done_v[c - NG])
        nrows = nb * 128
        col0 = (c * PLEN + b0) * 128 // 16
        nc.gpsimd.dma_gather(
            gout[c % NG][:, b0:b0 + nb, :],
            tab[:],
            idx_t[:, col0:col0 + nrows // 16],
            nrows,
            nrows,
            SD,
            single_packet=False,
            queue_num=1 + k % 3,
        ).then_inc(dsem[1 + k % 3], 16)

    # --- ACT: relu/square/sqrt stream ---------------------------------
    nx = 0

    def act(inst_fn):
        # ACT also executes in order; self-waits dropped (see dve()).
        nonlocal nx
        inst_fn().then_inc(xsem, 1)
        nx += 1

    nc.scalar.wait_ge(iosem, 48)
    for i in range(N_OUT):
        nc.scalar.wait_ge(vsem, q_done_v[i])
        act(lambda: nc.scalar.activation(
            out=ubuf[:], in_=qbuf[i % NQ][:],
            func=mybir.ActivationFunctionType.Relu,
            bias=c_t[:, i:i + 1]))
        act(lambda: nc.scalar.activation(
            out=sqb[:], in_=ubuf[:],
            func=mybir.ActivationFunctionType.Square,
            accum_out=ssb[:]))
        act(lambda: nc.scalar.activation(
            out=rt_all[:, i:i + 1], in_=ssb[:],
            func=mybir.ActivationFunctionType.Sqrt,
            scale=0.25))

    # --- store + end-of-kernel ----------------------------------------
    nc.sync.wait_ge(xsem, nx)
    nc.sync.dma_start(out=out[:], in_=rt_all[:]).then_inc(osem, 16)
    nc.sync.wait_ge(osem, 16)
    for s in all_sems:
        nc.sync.sem_clear(s)

    nc.finalize()
    return nc


def _host_prep(node_embedding, pos_path, neg_path):
    """Fold+quantize the table; per-core dedup working set + int16 indices;
    per-pair bias c[b,n]."""
    pos = np.asarray(pos_path).astype(np.int64)
    neg = np.asarray(neg_path).astype(np.int64)
    p1, p2 = pos[:, 0], pos[:, 1]

    inter_pos = (p1[:, :, None] == p2[:, None, :]).any(-1).sum(-1)
    diff_pos = np.maximum(PLEN - inter_pos, 1).astype(np.float32)
    inter_neg = (p1[:, None, :, None] == neg[:, :, None, :]).any(-1).sum(-1)
    diff_neg_raw = (PLEN - inter_neg).astype(np.float32)
    k = diff_neg_raw - 1.0
    diff_neg = np.maximum(diff_neg_raw, 1.0)
    # device consumes 2c (the 0.5 tmp scale is folded into the final sqrt)
    c = (2.0 * SCR * (k * CIRCLE_MARGIN + diff_pos[:, None] - diff_neg)).astype(
        np.float32
    )

    emb = np.asarray(node_embedding, dtype=np.float32)
    folded16 = (emb[:, :SD] + emb[:, SD:]).astype(ml_dtypes.bfloat16)

    in_maps = []
    for core in range(N_CORES):
        b0 = core * BPC
        # gathered row ids in order i = (chunk*8 + j)*128 + p
        rows = np.empty((N_CHUNK, PLEN, 128), dtype=np.int64)
        c_arr = np.empty((128, N_OUT), dtype=np.float32)
        for bt in range(NBT):
            bsl = slice(b0 + bt * 128, b0 + (bt + 1) * 128)
            rows[bt] = p2[bsl].T                      # p2 chunk: [j, p]
            for n in range(NNEG):
                rows[NBT + bt * NNEG + n] = neg[bsl, n, :].T
            c_arr[:, bt * NNEG:(bt + 1) * NNEG] = c[bsl]
        flat = rows.reshape(-1)
        uniq, inv = np.unique(flat, return_inverse=True)
        assert len(uniq) <= N_ROWS
        tab = np.zeros((N_ROWS, SD), dtype=ml_dtypes.bfloat16)
        tab[: len(uniq)] = folded16[uniq]
        inv16 = inv.astype(np.int16)
        # wrap for dma_gather: flat i -> partition i%16, col i//16, x8 groups
        idx_arr = np.tile(
            inv16.reshape(N_ROWS // 16, 16).T, (8, 1)
        )  # [128, N_ROWS//16]
        in_maps.append({"tab": tab, "idx": idx_arr, "cbias": c_arr})
    return in_maps


def kernel(node_embedding, pos_path, neg_path):
    if "nc" not in _CACHE:
        _CACHE["nc"] = _build_nc()
    nc = _CACHE["nc"]
    in_maps = _host_prep(node_embedding, pos_path, neg_path)
    res = run_bass_kernel_spmd(nc, in_maps, list(range(N_CORES)))
    _CACHE["last_result"] = res
    total = np.float64(0.0)
    for core in range(N_CORES):
        total += np.asarray(res.results[core]["out"], dtype=np.float64).sum()
    return np.array([total], dtype=np.float32)
